# revision 2
# baseline (speedup 1.0000x reference)
"""Trainium2 Bass kernel for CrossNetGatingMixLayer.

Math (per layer i, with U,C,V per expert e; gate = softmax over a singleton
axis == 1.0 identically, so the gating einsum and G are dead code):

    xv = tanh(xl @ V[e])          (B,R)  per expert
    xc = tanh(xv @ C[e].T)        (B,R)
    xu = xc @ U[e].T              (B,D)
    xl = xl + x0 * (sum_e xu + E * bias)

Strategy: data-parallel over 8 NeuronCores (batch split 16384 -> 8 x 2048).
On-chip layout is transposed ([d, b]): all matmuls contract over d or r with
the contraction dim on SBUF partitions.  Matmuls run in float32r (4x faster
than fp32 on the PE; inputs rounded to 11 mantissa bits) while the residual
stream xl stays fp32.  x is transposed in/out via PE-transpose.
"""
import numpy as np
from contextlib import ExitStack

import concourse.bass as bass
from concourse import bacc
import concourse.mybir as mybir
import concourse.tile as tile
from concourse.bass_utils import run_bass_kernel_spmd
from concourse.masks import make_identity

B, D, L, E, R = 16384, 512, 3, 4, 128
NCORES = 8
BL = B // NCORES            # 2048 rows per core
NBT = BL // 128             # 16 batch tiles of 128
NBC = BL // 512             # 4 batch chunks of 512 (matmul free dim)
ND = D // 128               # 4 d-chunks of 128
f32 = mybir.dt.float32
f32r = mybir.dt.float32r
Tanh = mybir.ActivationFunctionType.Tanh

_prog_cache = {}


def _build(has_bias: bool, use_f32r: bool):
    mmdt = f32r if use_f32r else f32
    nc = bacc.Bacc("TRN2")
    x_d = nc.declare_dram_parameter("x", [BL, D], f32, isOutput=False)
    Vs_d = nc.declare_dram_parameter("Vs", [L, E, D, R], f32, isOutput=False)
    Cs_d = nc.declare_dram_parameter("Cs", [L, E, R, R], f32, isOutput=False)
    Us_d = nc.declare_dram_parameter("Us", [L, E, D, R], f32, isOutput=False)
    if has_bias:
        b_d = nc.declare_dram_parameter("b", [L, D], f32, isOutput=False)
    out_d = nc.declare_dram_parameter("out", [BL, D], f32, isOutput=True)

    with tile.TileContext(nc) as tc, ExitStack() as ctx:
        const = ctx.enter_context(tc.tile_pool(name="const", bufs=1))
        wpool = ctx.enter_context(tc.tile_pool(name="wpool", bufs=1))
        xpool = ctx.enter_context(tc.tile_pool(name="xpool", bufs=1))
        wtmp_p = ctx.enter_context(tc.tile_pool(name="wtmp_p", bufs=3))
        ptr = ctx.enter_context(tc.tile_pool(name="ptr", bufs=2, space="PSUM"))
        ph_p = ctx.enter_context(tc.tile_pool(name="ph_p", bufs=2, space="PSUM"))
        pz_p = ctx.enter_context(tc.tile_pool(name="pz_p", bufs=2, space="PSUM"))
        pu_p = ctx.enter_context(tc.tile_pool(name="pu_p", bufs=2, space="PSUM"))

        ident = const.tile([128, 128], f32)
        make_identity(nc, ident)

        # ---- persistent weight tiles (mmdt) ----
        Vr = wpool.tile([128, L * E * ND, R], mmdt)        # V[l,e] d-chunk kd: [d128, r128]
        Cr = wpool.tile([128, L * E, R], mmdt)             # C[l,e].T: [s128, r128]
        Ur = wpool.tile([128, L * E, ND, 128], mmdt)       # U[l,e].T d-chunk: [r128, d128]

        # V: natural layout already correct for lhsT
        if use_f32r:
            for le in range(L * E):
                l, e = divmod(le, E)
                vtmp = wtmp_p.tile([128, ND, R], f32, name=f"vtmp{le}", tag="wtmp")
                nc.sync.dma_start(
                    out=vtmp,
                    in_=Vs_d[l, e].rearrange("(kd p) r -> p kd r", p=128))
                nc.vector.tensor_copy(
                    Vr[:, le * ND:(le + 1) * ND, :], vtmp)
        else:
            nc.sync.dma_start(
                out=Vr.rearrange("p (le kd) r -> p le kd r", le=L * E),
                in_=Vs_d[:].rearrange("l e (kd p) r -> p (l e) kd r", p=128))

        # C: load natural [r, s], PE-transpose -> [s, r]
        for le in range(L * E):
            l, e = divmod(le, E)
            ctmp = wtmp_p.tile([128, R], f32, name=f"ctmp{le}", tag="wtmp")
            nc.sync.dma_start(out=ctmp, in_=Cs_d[l, e])
            pct = ptr.tile([128, 128], f32, name=f"pct{le}", tag="tr")
            nc.tensor.transpose(pct, ctmp, ident)
            nc.any.tensor_copy(Cr[:, le, :], pct)

        # U: load natural [d, r] chunks, PE-transpose -> [r, d-chunk]
        for le in range(L * E):
            l, e = divmod(le, E)
            utmp = wtmp_p.tile([128, ND, R], f32, name=f"utmp{le}", tag="wtmp")
            nc.sync.dma_start(
                out=utmp,
                in_=Us_d[l, e].rearrange("(kd p) r -> p kd r", p=128))
            for kd in range(ND):
                put = ptr.tile([128, 128], f32, name=f"put{le}_{kd}", tag="tr")
                nc.tensor.transpose(put, utmp[:, kd, :], ident)
                nc.any.tensor_copy(Ur[:, le, kd, :], put)

        if has_bias:
            btmp = wtmp_p.tile([1, L, D], f32, name="btmp", tag="bias")
            nc.sync.dma_start(out=btmp, in_=b_d[:].rearrange("l d -> 1 l d"))
            bias4 = wpool.tile([1, L, D], mmdt)
            nc.vector.tensor_scalar_mul(bias4, btmp, float(E))
            ones_t = wtmp_p.tile([1, 512], f32, name="ones_t", tag="bias")
            nc.vector.memset(ones_t, 1.0)
            ones_r = wpool.tile([1, 512], mmdt)
            nc.vector.tensor_copy(ones_r, ones_t)

        # ---- x: natural load + PE transpose into [d, b] layout ----
        xlT = xpool.tile([128, ND, BL], f32)      # residual stream, fp32
        x0r = xpool.tile([128, ND, BL], mmdt)     # original x, matmul dtype
        with tc.tile_pool(name="xnat_p", bufs=1) as xnat_p:
            xnat = xnat_p.tile([128, NBT, D], f32)
            nc.sync.dma_start(
                out=xnat, in_=x_d[:].rearrange("(t p) d -> p t d", p=128))
            for dc in range(ND):
                for t in range(NBT):
                    pxt = ptr.tile([128, 128], f32, name=f"pxt{dc}_{t}",
                                   tag="tr")
                    nc.tensor.transpose(
                        pxt, xnat[:, t, 128 * dc:128 * (dc + 1)], ident)
                    nc.any.tensor_copy(
                        xlT[:, dc, 128 * t:128 * (t + 1)], pxt)
                    nc.any.tensor_copy(
                        x0r[:, dc, 128 * t:128 * (t + 1)], pxt)

        # ---- main layer loop ----
        hz_p = ctx.enter_context(tc.tile_pool(name="hz_p", bufs=1))
        tmp_p = ctx.enter_context(tc.tile_pool(name="tmp_p", bufs=4))
        xlr_p = ctx.enter_context(tc.tile_pool(name="xlr_p", bufs=2))

        for l in range(L):
            for c in range(NBC):
                cols = slice(512 * c, 512 * (c + 1))
                if l == 0:
                    rhs1 = x0r
                    rcols = cols
                elif use_f32r:
                    xlr = xlr_p.tile([128, ND, 512], f32r,
                                     name=f"xlr{l}_{c}", tag="xlr")
                    nc.vector.tensor_copy(xlr, xlT[:, :, cols])
                    rhs1 = xlr
                    rcols = slice(0, 512)
                else:
                    rhs1 = xlT
                    rcols = cols

                zr = []
                for e in range(E):
                    le = l * E + e
                    ph = ph_p.tile([128, 512], f32, name=f"ph{l}_{c}_{e}",
                                   tag="ph")
                    for kd in range(ND):
                        nc.tensor.matmul(
                            ph,
                            lhsT=Vr[:, le * ND + kd, :],
                            rhs=rhs1[:, kd, rcols],
                            start=(kd == 0), stop=(kd == ND - 1))
                    hr = hz_p.tile([128, 512], mmdt, name=f"h{l}_{c}_{e}",
                                   tag="h", bufs=4)
                    nc.scalar.activation(hr, ph, Tanh)

                    pz = pz_p.tile([128, 512], f32, name=f"pz{l}_{c}_{e}",
                                   tag="pz")
                    nc.tensor.matmul(pz, lhsT=Cr[:, le, :], rhs=hr,
                                     start=True, stop=True)
                    z = hz_p.tile([128, 512], mmdt, name=f"z{l}_{c}_{e}",
                                  tag="z", bufs=8)
                    nc.scalar.activation(z, pz, Tanh)
                    zr.append(z)

                for dc in range(ND):
                    pu = pu_p.tile([128, 512], f32, name=f"pu{l}_{c}_{dc}",
                                   tag="pu")
                    for e in range(E):
                        le = l * E + e
                        nc.tensor.matmul(
                            pu, lhsT=Ur[:, le, dc, :], rhs=zr[e],
                            start=(e == 0),
                            stop=(e == E - 1 and not has_bias))
                    if has_bias:
                        nc.tensor.matmul(
                            pu, lhsT=bias4[:, l, 128 * dc:128 * (dc + 1)],
                            rhs=ones_r, start=False, stop=True)
                    tmp = tmp_p.tile([128, 512], f32, name=f"tmp{l}_{c}_{dc}",
                                     tag="tmp")
                    nc.vector.tensor_mul(
                        tmp, pu, x0r[:, dc, cols].bitcast(f32))
                    nc.vector.tensor_add(
                        xlT[:, dc, cols], xlT[:, dc, cols], tmp)

        # ---- final transpose back to natural + store ----
        onat_p = ctx.enter_context(tc.tile_pool(name="onat_p", bufs=4))
        for t in range(NBT):
            onat = onat_p.tile([128, D], f32, name=f"onat{t}", tag="onat")
            for dc in range(ND):
                pot = ptr.tile([128, 128], f32, name=f"pot{t}_{dc}", tag="tr")
                nc.tensor.transpose(
                    pot, xlT[:, dc, 128 * t:128 * (t + 1)], ident)
                nc.any.tensor_copy(onat[:, 128 * dc:128 * (dc + 1)], pot)
            nc.sync.dma_start(
                out=out_d[:].rearrange("(t p) d -> p t d", p=128)[:, t, :],
                in_=onat)

    nc.finalize()
    return nc


def _get_prog(has_bias: bool, use_f32r: bool = True):
    key = (has_bias, use_f32r)
    if key not in _prog_cache:
        _prog_cache[key] = _build(has_bias, use_f32r)
    return _prog_cache[key]


def _run(inputs, trace=False, use_f32r=True):
    x = np.ascontiguousarray(np.asarray(inputs["x"], dtype=np.float32))
    Us = np.ascontiguousarray(np.asarray(inputs["Us"], dtype=np.float32))
    Cs = np.ascontiguousarray(np.asarray(inputs["Cs"], dtype=np.float32))
    Vs = np.ascontiguousarray(np.asarray(inputs["Vs"], dtype=np.float32))
    b = np.ascontiguousarray(np.asarray(inputs["b"], dtype=np.float32))
    assert x.shape == (B, D), x.shape
    has_bias = bool(np.any(b))
    nc = _get_prog(has_bias, use_f32r)
    shards = np.split(x, NCORES, axis=0)
    in_maps = []
    for i in range(NCORES):
        m = {"x": shards[i], "Us": Us, "Cs": Cs, "Vs": Vs}
        if has_bias:
            m["b"] = b
        in_maps.append(m)
    res = run_bass_kernel_spmd(nc, in_maps, core_ids=list(range(NCORES)),
                               trace=trace)
    out = np.concatenate([res.results[i]["out"] for i in range(NCORES)],
                         axis=0)
    return out, res


def kernel(**inputs) -> np.ndarray:
    out, _ = _run(inputs)
    return out


# revision 10
# speedup vs baseline: 87.9113x; 87.9113x over previous
"""Trainium2 Bass kernel for CrossNetGatingMixLayer.

Math (per layer i, with U,C,V per expert e; gate = softmax over a singleton
axis == 1.0 identically, so the gating einsum and G are dead code):

    xv = tanh(xl @ V[e])          (B,R)  per expert
    xc = tanh(xv @ C[e].T)        (B,R)
    xu = xc @ U[e].T              (B,D)
    xl = xl + x0 * (sum_e xu + E * bias)

Strategy: data-parallel over 8 NeuronCores (batch split 16384 -> 8 x 2048).
On-chip layout is transposed ([d, b]): all matmuls contract over d or r with
the contraction dim on SBUF partitions.  Matmuls run in float32r (4x faster
than fp32 on the PE; inputs rounded to 11 mantissa bits) while the residual
stream xl stays fp32.  x is transposed in/out via PE-transpose, batched in
groups of four 128x128 blocks per PSUM tile so eviction copies are wide.
"""
import numpy as np
from contextlib import ExitStack

import concourse.bass as bass
from concourse import bacc
import concourse.mybir as mybir
import concourse.tile as tile
from concourse.bass_utils import run_bass_kernel_spmd
from concourse.masks import make_identity

B, D, L, E, R = 16384, 512, 3, 4, 128
NCORES = 8
BL = B // NCORES            # 2048 rows per core
NBT = BL // 128             # 16 batch tiles of 128
NBC = BL // 512             # 4 batch chunks of 512 (matmul free dim)
ND = D // 128               # 4 d-chunks of 128
f32 = mybir.dt.float32
f32r = mybir.dt.float32r
Tanh = mybir.ActivationFunctionType.Tanh

_prog_cache = {}


def _build(has_bias: bool, use_f32r: bool):
    mmdt = f32r if use_f32r else f32
    nc = bacc.Bacc("TRN2")
    x_d = nc.declare_dram_parameter("x", [BL, D], f32, isOutput=False)
    Vs_d = nc.declare_dram_parameter("Vs", [L, E, D, R], f32, isOutput=False)
    Cs_d = nc.declare_dram_parameter("Cs", [L, E, R, R], f32, isOutput=False)
    Us_d = nc.declare_dram_parameter("Us", [L, E, D, R], f32, isOutput=False)
    if has_bias:
        b_d = nc.declare_dram_parameter("b", [L, D], f32, isOutput=False)
    out_d = nc.declare_dram_parameter("out", [BL, D], f32, isOutput=True)

    with tile.TileContext(nc) as tc, ExitStack() as ctx:
        const = ctx.enter_context(tc.tile_pool(name="const", bufs=1))
        wpool = ctx.enter_context(tc.tile_pool(name="wpool", bufs=1))
        xpool = ctx.enter_context(tc.tile_pool(name="xpool", bufs=1))
        wtmp_p = ctx.enter_context(tc.tile_pool(name="wtmp_p", bufs=2))
        ptr = ctx.enter_context(tc.tile_pool(name="ptr", bufs=2, space="PSUM"))
        ph_p = ctx.enter_context(tc.tile_pool(name="ph_p", bufs=3, space="PSUM"))
        pz_p = ctx.enter_context(tc.tile_pool(name="pz_p", bufs=1, space="PSUM"))
        pu_p = ctx.enter_context(tc.tile_pool(name="pu_p", bufs=2, space="PSUM"))

        ident = const.tile([128, 128], f32)
        make_identity(nc, ident)

        # ---- persistent weight tiles (mmdt) ----
        Vr = wpool.tile([128, L, E, ND, R], mmdt)    # V[l,e] kd-chunk: [d128, r128]
        Cr = wpool.tile([128, L, E, R], mmdt)        # C[l,e].T: [s128, r128]
        Ur = wpool.tile([128, L, E, ND, 128], mmdt)  # U[l,e].T kd-chunk: [r128, d128]

        def prep_V(l):
            vtmp = wtmp_p.tile([128, E, ND, R], f32, name=f"vtmp{l}", tag="wtmp")
            nc.gpsimd.dma_start(
                out=vtmp,
                in_=Vs_d[l].rearrange("e (kd p) r -> p e kd r", p=128))
            nc.any.tensor_copy(Vr[:, l], vtmp)

        def prep_U(l):
            # U: [d, r] -> PE transpose to [r, d] chunks, batched 4-wide
            utmp = wtmp_p.tile([128, E, ND, R], f32, name=f"utmp{l}", tag="wtmp")
            nc.gpsimd.dma_start(
                out=utmp,
                in_=Us_d[l].rearrange("e (kd p) r -> p e kd r", p=128))
            for e in range(E):
                put = ptr.tile([128, 512], f32, name=f"put{l}_{e}", tag="tr")
                for kd in range(ND):
                    nc.tensor.transpose(
                        put[:, 128 * kd:128 * (kd + 1)], utmp[:, e, kd, :],
                        ident)
                nc.any.tensor_copy(
                    Ur[:, l, e].rearrange("p a b -> p (a b)"), put)

        def prep_C(l):
            # C: [r, s] -> [s, r], 4 experts batched into one psum tile
            ctmp = wtmp_p.tile([128, E, R], f32, name=f"ctmp{l}", tag="wtmp")
            nc.gpsimd.dma_start(out=ctmp, in_=Cs_d[l].rearrange("e r s -> r e s"))
            pct = ptr.tile([128, 512], f32, name=f"pct{l}", tag="tr")
            for e in range(E):
                nc.tensor.transpose(
                    pct[:, 128 * e:128 * (e + 1)], ctmp[:, e, :], ident)
            nc.any.tensor_copy(Cr[:, l].rearrange("p a b -> p (a b)"), pct)

        if has_bias:
            btmp = wtmp_p.tile([1, L, D], f32, name="btmp", tag="bias")
            nc.sync.dma_start(out=btmp, in_=b_d[:].rearrange("l d -> 1 l d"))
            bias4 = wpool.tile([1, L, D], mmdt)
            nc.vector.tensor_scalar_mul(bias4, btmp, float(E))
            ones_t = wtmp_p.tile([1, 512], f32, name="ones_t", tag="bias")
            nc.vector.memset(ones_t, 1.0)
            ones_r = wpool.tile([1, 512], mmdt)
            nc.vector.tensor_copy(ones_r, ones_t)

        # ---- x: natural load + PE transpose into [d, b] layout ----
        # Order: V(l=0) first so mm1 can start as soon as batch-group g=0 is
        # transposed; group-major transpose order so chunk c only needs the
        # first c+1 groups; x0r copied per group straight from PSUM.
        xlT = xpool.tile([128, ND, BL], f32)      # residual stream, fp32
        x0r = xpool.tile([128, ND, BL], mmdt)     # original x, matmul dtype
        with tc.tile_pool(name="xnat_p", bufs=1) as xnat_p:
            xnat = xnat_p.tile([128, NBT, D], f32)
            for t in range(4):
                nc.sync.dma_start(
                    out=xnat[:, t, :],
                    in_=x_d[128 * t:128 * (t + 1), :])
            prep_V(0)
            prep_C(0)
            for t in range(4, NBT):
                nc.sync.dma_start(
                    out=xnat[:, t, :],
                    in_=x_d[128 * t:128 * (t + 1), :])
            for g in range(NBT // 4):
                for dc in range(ND):
                    pxt = ptr.tile([128, 512], f32, name=f"pxt{dc}_{g}",
                                   tag="tr")
                    for i in range(4):
                        nc.tensor.transpose(
                            pxt[:, 128 * i:128 * (i + 1)],
                            xnat[:, 4 * g + i, 128 * dc:128 * (dc + 1)],
                            ident)
                    nc.any.tensor_copy(
                        xlT[:, dc, 512 * g:512 * (g + 1)], pxt)
                    nc.any.tensor_copy(
                        x0r[:, dc, 512 * g:512 * (g + 1)], pxt)
                if g == 0:
                    prep_U(0)
                elif g == 1:
                    prep_V(1)
                    prep_C(1)
                elif g == 2:
                    prep_U(1)
                elif g == 3:
                    prep_V(2)
                    prep_C(2)
                    prep_U(2)

        # ---- main layer loop ----
        hz_p = ctx.enter_context(tc.tile_pool(name="hz_p", bufs=1))
        tmp_p = ctx.enter_context(tc.tile_pool(name="tmp_p", bufs=6))
        xlr_p = ctx.enter_context(tc.tile_pool(name="xlr_p", bufs=3))
        onat_p = ctx.enter_context(tc.tile_pool(name="onat_p", bufs=3))

        for l in range(L):
            for c in range(NBC):
                cols = slice(512 * c, 512 * (c + 1))
                if l == 0:
                    rhs1 = x0r
                    rcols = cols
                elif use_f32r:
                    xlr = xlr_p.tile([128, ND, 512], f32r,
                                     name=f"xlr{l}_{c}", tag="xlr")
                    for dc in range(ND):
                        nc.any.tensor_copy(xlr[:, dc, :], xlT[:, dc, cols])
                    rhs1 = xlr
                    rcols = slice(0, 512)
                else:
                    rhs1 = xlT
                    rcols = cols

                zr = []
                for e in range(E):
                    ph = ph_p.tile([128, 512], f32, name=f"ph{l}_{c}_{e}",
                                   tag="ph")
                    for kd in range(ND):
                        nc.tensor.matmul(
                            ph,
                            lhsT=Vr[:, l, e, kd, :],
                            rhs=rhs1[:, kd, rcols],
                            start=(kd == 0), stop=(kd == ND - 1))
                    hr = hz_p.tile([128, 512], mmdt, name=f"h{l}_{c}_{e}",
                                   tag="h", bufs=6)
                    nc.scalar.activation(hr, ph, Tanh)

                    pz = pz_p.tile([128, 512], f32, name=f"pz{l}_{c}_{e}",
                                   tag="pz")
                    nc.tensor.matmul(pz, lhsT=Cr[:, l, e, :], rhs=hr,
                                     start=True, stop=True)
                    z = hz_p.tile([128, 512], mmdt, name=f"z{l}_{c}_{e}",
                                  tag="z", bufs=8)
                    nc.scalar.activation(z, pz, Tanh)
                    zr.append(z)

                for dc in range(ND):
                    pu = pu_p.tile([128, 512], f32, name=f"pu{l}_{c}_{dc}",
                                   tag="pu")
                    for e in range(E):
                        nc.tensor.matmul(
                            pu, lhsT=Ur[:, l, e, dc, :], rhs=zr[e],
                            start=(e == 0),
                            stop=(e == E - 1 and not has_bias))
                    if has_bias:
                        nc.tensor.matmul(
                            pu, lhsT=bias4[:, l, 128 * dc:128 * (dc + 1)],
                            rhs=ones_r, start=False, stop=True)
                    tmp = tmp_p.tile([128, 512], f32, name=f"tmp{l}_{c}_{dc}",
                                     tag="tmp")
                    nc.vector.tensor_mul(
                        tmp, pu, x0r[:, dc, cols].bitcast(f32))
                    nc.vector.tensor_add(
                        xlT[:, dc, cols], xlT[:, dc, cols], tmp)

                if l == L - 1:
                    # store this chunk: transpose back to natural + DMA out
                    for t in range(4 * c, 4 * (c + 1)):
                        pot = ptr.tile([128, 512], f32, name=f"pot{t}",
                                       tag="tr")
                        for dc in range(ND):
                            nc.tensor.transpose(
                                pot[:, 128 * dc:128 * (dc + 1)],
                                xlT[:, dc, 128 * t:128 * (t + 1)], ident)
                        onat = onat_p.tile([128, D], f32, name=f"onat{t}",
                                           tag="onat")
                        nc.any.tensor_copy(onat, pot)
                        nc.sync.dma_start(
                            out=out_d[128 * t:128 * (t + 1), :], in_=onat)


    nc.finalize()
    return nc


def _get_prog(has_bias: bool, use_f32r: bool = True):
    key = (has_bias, use_f32r)
    if key not in _prog_cache:
        _prog_cache[key] = _build(has_bias, use_f32r)
    return _prog_cache[key]


def _run(inputs, trace=False, use_f32r=True):
    x = np.ascontiguousarray(np.asarray(inputs["x"], dtype=np.float32))
    Us = np.ascontiguousarray(np.asarray(inputs["Us"], dtype=np.float32))
    Cs = np.ascontiguousarray(np.asarray(inputs["Cs"], dtype=np.float32))
    Vs = np.ascontiguousarray(np.asarray(inputs["Vs"], dtype=np.float32))
    b = np.ascontiguousarray(np.asarray(inputs["b"], dtype=np.float32))
    assert x.shape == (B, D), x.shape
    has_bias = bool(np.any(b))
    nc = _get_prog(has_bias, use_f32r)
    shards = np.split(x, NCORES, axis=0)
    in_maps = []
    for i in range(NCORES):
        m = {"x": shards[i], "Us": Us, "Cs": Cs, "Vs": Vs}
        if has_bias:
            m["b"] = b
        in_maps.append(m)
    res = run_bass_kernel_spmd(nc, in_maps, core_ids=list(range(NCORES)),
                               trace=trace)
    out = np.concatenate([res.results[i]["out"] for i in range(NCORES)],
                         axis=0)
    return out, res


def kernel(**inputs) -> np.ndarray:
    out, _ = _run(inputs)
    return out
